# revision 6
# baseline (speedup 1.0000x reference)
"""Fused biased-softmax attention (nn_Attention_55576876810478) on 8 TRN2 NeuronCores.

Tensor-parallel by head (H=8 -> 1 head/core): core h computes head h end to
end -- q/k/v/gate projections, scores with bias_mask+bias_pair, softmax,
P@V, sigmoid gate, and its 32 rows of the output projection -- producing a
partial [B*Q, D] output.  The host sums the 8 partials (the "all-reduce
after linear_o" of the sharding hint, done during unshard) and adds bo.

On-chip layout choices:
  * scores are computed TRANSPOSED, S^T[k, q], so the P@V contraction (over
    k) lands on the partition axis, and bias_mask (a function of k only)
    becomes a per-partition bias folded into the ACT exp instruction.
  * bias_pair arrives host-pre-transposed as bpT[b, kt, k%128, q] (bf16) and
    is accumulated into the scores PSUM with an identity matmul.
  * softmax denominators come for free from the P@V matmul: the stationary
    operand is [V | ones-columns], so row 32+t of the PV accumulator is
    sum_k P[k, q].  Denominators are transposed to [q-partition, 1] columns
    with a tiny K=8 identity matmul, and the divide is applied as a
    per-partition tensor_scalar multiply while evacuating the final matmul.
"""

import math

import ml_dtypes
import numpy as np

B, Q, KL, D, H, C = 4, 1024, 1024, 256, 8, 32
NCORES = 8
BQ = B * Q            # 4096 flattened q positions
BK = B * KL           # 4096 flattened k positions
QT = 512              # q-tile width (free dim of S^T)
KT = 128              # k-tile height (partition dim of S^T)
NQT = BQ // QT        # 8 (b, jq) q-tiles
NKT = KL // KT        # 8 k-tiles per batch
NVG = BK // KT        # 32 global k-tiles (v projection)
NF = BQ // 128        # 32 final output row-tiles

_BF16 = ml_dtypes.bfloat16
_CACHE = {}


def _build_nc():
    import concourse.bass as bass  # noqa: F401
    import concourse.mybir as mybir
    import concourse.tile as tile
    from concourse.bacc import Bacc

    bf16 = mybir.dt.bfloat16
    f32 = mybir.dt.float32
    AF = mybir.ActivationFunctionType
    ALU = mybir.AluOpType

    nc = Bacc(None, target_bir_lowering=False)

    qxT_d = nc.dram_tensor("qxT", [2, 128, BQ], bf16, kind="ExternalInput")
    kvxT_d = nc.dram_tensor("kvxT", [2, 128, BK], bf16, kind="ExternalInput")
    bpT_d = nc.dram_tensor("bpT", [B, NKT, KT, Q], bf16, kind="ExternalInput")
    bm_d = nc.dram_tensor("bm", [128, B * NKT], f32, kind="ExternalInput")
    wq_d = nc.dram_tensor("wq", [2, 128, C], bf16, kind="ExternalInput")
    wk_d = nc.dram_tensor("wk", [2, 128, C], bf16, kind="ExternalInput")
    wv_d = nc.dram_tensor("wv", [2, 128, C], bf16, kind="ExternalInput")
    wg_d = nc.dram_tensor("wg", [2, 128, C], bf16, kind="ExternalInput")
    bg_d = nc.dram_tensor("bg", [C, 1], f32, kind="ExternalInput")
    wo_d = nc.dram_tensor("wo", [C, D], bf16, kind="ExternalInput")
    id128_d = nc.dram_tensor("id128", [128, 128], bf16, kind="ExternalInput")
    out_d = nc.dram_tensor("out", [BQ, D], f32, kind="ExternalOutput")

    with tile.TileContext(nc) as tc:
        with (
            tc.tile_pool(name="const", bufs=1) as const,
            tc.tile_pool(name="proj", bufs=1) as proj,
            tc.tile_pool(name="biasp", bufs=3) as biasp,
            tc.tile_pool(name="pp", bufs=3) as pp,
            tc.tile_pool(name="outp", bufs=3) as outp,
            tc.tile_pool(name="ps_s", bufs=2, space="PSUM") as ps_s,
            tc.tile_pool(name="ps_pv", bufs=2, space="PSUM") as ps_pv,
            tc.tile_pool(name="ps_m", bufs=2, space="PSUM") as ps_m,
        ):
            # ---------------- constants / inputs ----------------
            qxT = const.tile([128, 2, BQ], bf16)
            kvxT = const.tile([128, 2, BK], bf16)
            for dc in range(2):
                nc.sync.dma_start(qxT[:, dc, :], qxT_d[dc])
                nc.sync.dma_start(kvxT[:, dc, :], kvxT_d[dc])
            wq = const.tile([128, 2, C], bf16)
            wk = const.tile([128, 2, C], bf16)
            wv = const.tile([128, 2, C], bf16)
            wg = const.tile([128, 2, C], bf16)
            for sb, dr in ((wq, wq_d), (wk, wk_d), (wv, wv_d), (wg, wg_d)):
                for dc in range(2):
                    nc.sync.dma_start(sb[:, dc, :], dr[dc])
            bm = const.tile([128, B * NKT], f32)
            nc.sync.dma_start(bm, bm_d[:, :])
            bg = const.tile([C, 1], f32)
            nc.sync.dma_start(bg, bg_d[:, :])
            wo = const.tile([C, D], bf16)
            nc.sync.dma_start(wo, wo_d[:, :])
            id128 = const.tile([128, 128], bf16)
            nc.sync.dma_start(id128, id128_d[:, :])

            # persistent intermediates
            qT = proj.tile([C, BQ], bf16)        # [c, b*Q+q]
            gT = proj.tile([33, BQ], bf16)       # sigmoid gate; row 32 = 1.0
            kT = proj.tile([C, BK], bf16)        # [c, b*K+k]
            vones = proj.tile([128, NVG, 33], bf16)  # [k%128, ktile, c|ones]
            odn = proj.tile([33, BQ], bf16)      # gated O^T (rows 0:32) + denom (row 32)
            den_cols = proj.tile([128, NF], f32)  # den_cols[p, f] = denom(q = p*32+f)
            recip = proj.tile([128, NF], f32)

            nc.vector.memset(vones, 1.0)
            nc.vector.memset(gT[32:33, :], 1.0)

            # ---------------- projections ----------------
            for j in range(NQT):
                sl = slice(j * QT, (j + 1) * QT)
                q_ps = ps_m.tile([C, QT], f32, tag="m")
                for dc in range(2):
                    nc.tensor.matmul(q_ps, wq[:, dc, :], qxT[:, dc, sl],
                                     start=dc == 0, stop=dc == 1)
                nc.vector.tensor_copy(qT[:, sl], q_ps)
                g_ps = ps_m.tile([C, QT], f32, tag="m")
                for dc in range(2):
                    nc.tensor.matmul(g_ps, wg[:, dc, :], qxT[:, dc, sl],
                                     start=dc == 0, stop=dc == 1)
                nc.scalar.activation(gT[0:C, sl], g_ps, AF.Sigmoid, bias=bg)
                k_ps = ps_m.tile([C, QT], f32, tag="m")
                for dc in range(2):
                    nc.tensor.matmul(k_ps, wk[:, dc, :], kvxT[:, dc, sl],
                                     start=dc == 0, stop=dc == 1)
                nc.vector.tensor_copy(kT[:, sl], k_ps)
            for g in range(NVG):
                v_ps = ps_m.tile([128, C], f32, tag="m")
                for dc in range(2):
                    nc.tensor.matmul(v_ps, kvxT[:, dc, g * KT:(g + 1) * KT],
                                     wv[:, dc, :], start=dc == 0, stop=dc == 1)
                nc.vector.tensor_copy(vones[:, g, 0:C], v_ps)

            # ---------------- attention ----------------
            for b in range(B):
                pv = [ps_pv.tile([33, QT], f32, tag="pv", name=f"pv_{b}_{i}")
                      for i in range(2)]
                for kt in range(NKT):
                    gk = b * NKT + kt
                    bp = biasp.tile([128, Q], bf16)
                    nc.sync.dma_start(bp, bpT_d[b, kt])
                    s = ps_s.tile([128, Q], f32, tag="s")
                    for jq in range(2):
                        ssl = s[:, jq * QT:(jq + 1) * QT]
                        nc.tensor.matmul(
                            ssl,
                            kT[:, b * KL + kt * KT: b * KL + (kt + 1) * KT],
                            qT[:, b * Q + jq * QT: b * Q + (jq + 1) * QT],
                            start=True, stop=False)
                        nc.tensor.matmul(
                            ssl, id128, bp[:, jq * QT:(jq + 1) * QT],
                            start=False, stop=True)
                    p = pp.tile([128, Q], bf16)
                    nc.scalar.activation(p, s, AF.Exp, bias=bm[:, gk:gk + 1])
                    for jq in range(2):
                        nc.tensor.matmul(
                            pv[jq][0:33, :], vones[:, gk, :],
                            p[:, jq * QT:(jq + 1) * QT],
                            start=kt == 0, stop=kt == NKT - 1)
                for jq in range(2):
                    qsl = slice(b * Q + jq * QT, b * Q + (jq + 1) * QT)
                    # odn = (pv * 1.0) * [gate; 1]  (fused evict + gate mult;
                    # row 32 passes the softmax denominator through)
                    nc.vector.scalar_tensor_tensor(
                        odn[:, qsl], pv[jq][0:33, :], 1.0,
                        gT[:, qsl], op0=ALU.mult, op1=ALU.mult)

            # ---------------- denominators -> per-partition columns ----------
            # SBUF->SBUF DMA walks dst (p, f) in order, consuming the source
            # row linearly: den_cols[p, f] = denom(q = p*32 + f).
            nc.gpsimd.dma_start(den_cols, odn[32:33, :])
            nc.vector.reciprocal(recip, den_cols)

            # ---------------- output projection ----------------
            # final tile f covers the strided q-set {p*32 + f}: lhsT columns
            # q = p*32+f, per-partition denominators recip[:, f], and a DRAM
            # AP with constant partition stride 32 rows.
            og_r = odn[0:C, :].rearrange("c (p f) -> c f p", f=NF)
            out_r = out_d[:, :].rearrange("(p f) d -> f p d", f=NF)
            for f in range(NF):
                fo = ps_m.tile([128, D], f32, tag="m")
                nc.tensor.matmul(fo, og_r[:, f, :], wo,
                                 start=True, stop=True)
                ot = outp.tile([128, D], f32)
                nc.vector.tensor_scalar_mul(ot, fo, recip[:, f:f + 1])
                nc.sync.dma_start(out_r[f], ot)

    nc.finalize()
    return nc


def _get_nc():
    if "nc" not in _CACHE:
        _CACHE["nc"] = _build_nc()
    return _CACHE["nc"]


def _prep(inputs):
    q_x = np.asarray(inputs["q_x"], np.float32)
    kv_x = np.asarray(inputs["kv_x"], np.float32)
    bias_mask = np.asarray(inputs["bias_mask"], np.float32)
    bias_pair = np.asarray(inputs["bias_pair"], np.float32)
    wq = np.asarray(inputs["wq"], np.float32)
    wk = np.asarray(inputs["wk"], np.float32)
    wv = np.asarray(inputs["wv"], np.float32)
    wg = np.asarray(inputs["wg"], np.float32)
    bg = np.asarray(inputs["bg"], np.float32)
    wo = np.asarray(inputs["wo"], np.float32)

    qxT = np.ascontiguousarray(q_x.reshape(BQ, D).T).astype(_BF16).reshape(2, 128, BQ)
    kvxT = np.ascontiguousarray(kv_x.reshape(BK, D).T).astype(_BF16).reshape(2, 128, BK)
    bm = np.ascontiguousarray(
        bias_mask.reshape(B, NKT, KT).transpose(2, 0, 1)).astype(np.float32)
    bm = bm.reshape(128, B * NKT)
    id128 = np.eye(128, dtype=_BF16)
    sc = 1.0 / math.sqrt(C)

    in_maps = []
    for h in range(NCORES):
        csl = slice(h * C, (h + 1) * C)
        bpT = np.ascontiguousarray(
            bias_pair[:, h].transpose(0, 2, 1)).astype(_BF16)
        bpT = bpT.reshape(B, NKT, KT, Q)
        in_maps.append({
            "qxT": qxT,
            "kvxT": kvxT,
            "bpT": bpT,
            "bm": bm,
            "wq": np.ascontiguousarray(wq[:, csl] * sc).astype(_BF16).reshape(2, 128, C),
            "wk": np.ascontiguousarray(wk[:, csl]).astype(_BF16).reshape(2, 128, C),
            "wv": np.ascontiguousarray(wv[:, csl]).astype(_BF16).reshape(2, 128, C),
            "wg": np.ascontiguousarray(wg[:, csl]).astype(_BF16).reshape(2, 128, C),
            "bg": np.ascontiguousarray(bg[csl]).astype(np.float32).reshape(C, 1),
            "wo": np.ascontiguousarray(wo[csl, :]).astype(_BF16),
            "id128": id128,
        })
    return in_maps


def _run(inputs, trace=False, **kw):
    from concourse.bass_utils import run_bass_kernel_spmd

    in_maps = _prep(inputs)
    nc = _get_nc()
    r = run_bass_kernel_spmd(nc, in_maps, core_ids=list(range(NCORES)),
                             trace=trace, **kw)
    bo = np.asarray(inputs["bo"], np.float32)
    total = np.zeros((BQ, D), np.float32)
    for i in range(NCORES):
        total += r.results[i]["out"].reshape(BQ, D).astype(np.float32)
    total += bo
    return total.reshape(B, Q, D).astype(np.float32), r


def kernel(**inputs):
    out, _ = _run(inputs, trace=False)
    return out


# revision 7
# speedup vs baseline: 1.0663x; 1.0663x over previous
"""Fused biased-softmax attention (nn_Attention_55576876810478) on 8 TRN2 NeuronCores.

Tensor-parallel by head (H=8 -> 1 head/core): core h computes head h end to
end -- q/k/v/gate projections, scores with bias_mask+bias_pair, softmax,
P@V, sigmoid gate, and its 32 rows of the output projection -- producing a
partial [B*Q, D] output.  The host sums the 8 partials (the "all-reduce
after linear_o" of the sharding hint, done during unshard) and adds bo.

On-chip layout choices:
  * scores are computed TRANSPOSED, S^T[k, q], so the P@V contraction (over
    k) lands on the partition axis, and bias_mask (a function of k only)
    becomes a per-partition bias folded into the ACT exp instruction.
  * bias_pair arrives host-pre-transposed as bpT[b, kt, k%128, q] (bf16) and
    is accumulated into the scores PSUM with an identity matmul.
  * softmax denominators come for free from the P@V matmul: the stationary
    operand is [V | ones-columns], so row 32+t of the PV accumulator is
    sum_k P[k, q].  Denominators are transposed to [q-partition, 1] columns
    with a tiny K=8 identity matmul, and the divide is applied as a
    per-partition tensor_scalar multiply while evacuating the final matmul.
"""

import math

import ml_dtypes
import numpy as np

B, Q, KL, D, H, C = 4, 1024, 1024, 256, 8, 32
NCORES = 8
BQ = B * Q            # 4096 flattened q positions
BK = B * KL           # 4096 flattened k positions
QT = 512              # q-tile width (free dim of S^T)
KT = 128              # k-tile height (partition dim of S^T)
NQT = BQ // QT        # 8 (b, jq) q-tiles
NKT = KL // KT        # 8 k-tiles per batch
NVG = BK // KT        # 32 global k-tiles (v projection)
NF = BQ // 128        # 32 final output row-tiles

_BF16 = ml_dtypes.bfloat16
_CACHE = {}


def _build_nc():
    import concourse.bass as bass  # noqa: F401
    import concourse.mybir as mybir
    import concourse.tile as tile
    from concourse.bacc import Bacc

    bf16 = mybir.dt.bfloat16
    f32 = mybir.dt.float32
    AF = mybir.ActivationFunctionType
    ALU = mybir.AluOpType

    nc = Bacc(None, target_bir_lowering=False)

    qxT_d = nc.dram_tensor("qxT", [2, 128, BQ], bf16, kind="ExternalInput")
    kvxT_d = nc.dram_tensor("kvxT", [2, 128, BK], bf16, kind="ExternalInput")
    bpT_d = nc.dram_tensor("bpT", [B, NKT, KT, Q], bf16, kind="ExternalInput")
    bm_d = nc.dram_tensor("bm", [128, B * NKT], f32, kind="ExternalInput")
    wqg_d = nc.dram_tensor("wqg", [2, 128, 2 * C], bf16, kind="ExternalInput")
    wk_d = nc.dram_tensor("wk", [2, 128, C], bf16, kind="ExternalInput")
    wv_d = nc.dram_tensor("wv", [2, 128, C], bf16, kind="ExternalInput")
    bg_d = nc.dram_tensor("bg", [2 * C, 1], f32, kind="ExternalInput")
    wo_d = nc.dram_tensor("wo", [C, D], bf16, kind="ExternalInput")
    id128_d = nc.dram_tensor("id128", [128, 128], bf16, kind="ExternalInput")
    out_d = nc.dram_tensor("out", [BQ, D], f32, kind="ExternalOutput")

    with tile.TileContext(nc) as tc:
        with (
            tc.tile_pool(name="const", bufs=1) as const,
            tc.tile_pool(name="proj", bufs=1) as proj,
            tc.tile_pool(name="biasp", bufs=5) as biasp,
            tc.tile_pool(name="pp", bufs=4) as pp,
            tc.tile_pool(name="outp", bufs=3) as outp,
        ):
            # ---------------- constants / inputs ----------------
            qxT = const.tile([128, 2, BQ], bf16)
            kvxT = const.tile([128, 2, BK], bf16)
            for dc in range(2):
                nc.sync.dma_start(qxT[:, dc, :], qxT_d[dc])
                nc.sync.dma_start(kvxT[:, dc, :], kvxT_d[dc])
            wqg = const.tile([128, 2, 2 * C], bf16)
            wk = const.tile([128, 2, C], bf16)
            wv = const.tile([128, 2, C], bf16)
            for sb, dr in ((wqg, wqg_d), (wk, wk_d), (wv, wv_d)):
                for dc in range(2):
                    nc.sync.dma_start(sb[:, dc, :], dr[dc])
            bm = const.tile([128, B * NKT], f32)
            nc.sync.dma_start(bm, bm_d[:, :])
            bg = const.tile([2 * C, 1], f32)
            nc.sync.dma_start(bg, bg_d[:, :])
            wo = const.tile([C, D], bf16)
            nc.sync.dma_start(wo, wo_d[:, :])
            id128 = const.tile([128, 128], bf16)
            nc.sync.dma_start(id128, id128_d[:, :])

            # persistent intermediates
            qT = proj.tile([C, BQ], bf16)        # [c, b*Q+q]
            ghi = proj.tile([2 * C, BQ], bf16)   # sigmoid out on partitions 32:64
            gT = proj.tile([33, BQ], bf16)       # sigmoid gate; row 32 = 1.0
            kT = proj.tile([C, BK], bf16)        # [c, b*K+k]
            vones = proj.tile([128, NVG, 33], bf16)  # [k%128, ktile, c|ones]
            odn = proj.tile([33, BQ], bf16)      # gated O^T (rows 0:32) + denom (row 32)
            den_cols = proj.tile([128, NF], f32)  # den_cols[p, f] = denom(q = p*32+f)
            recip = proj.tile([128, NF], f32)

            nc.vector.memset(vones, 1.0)
            nc.vector.memset(gT[32:33, :], 1.0)

            # ---------------- projections ----------------
            with tc.tile_pool(name="ps_proj", bufs=3, space="PSUM") as ps_pj:
                for j in range(NQT):
                    sl = slice(j * QT, (j + 1) * QT)
                    qg_ps = ps_pj.tile([2 * C, QT], f32, tag="m")
                    for dc in range(2):
                        nc.tensor.matmul(qg_ps, wqg[:, dc, :], qxT[:, dc, sl],
                                         start=dc == 0, stop=dc == 1)
                    nc.vector.tensor_copy(qT[:, sl], qg_ps[0:C, :])
                    nc.scalar.activation(ghi[C:2 * C, sl], qg_ps[C:2 * C, :],
                                         AF.Sigmoid, bias=bg[C:2 * C, :])
                    k_ps = ps_pj.tile([C, QT], f32, tag="m")
                    for dc in range(2):
                        nc.tensor.matmul(k_ps, wk[:, dc, :], kvxT[:, dc, sl],
                                         start=dc == 0, stop=dc == 1)
                    nc.vector.tensor_copy(kT[:, sl], k_ps)
                for g in range(NVG):
                    v_ps = ps_pj.tile([128, C], f32, tag="m")
                    for dc in range(2):
                        nc.tensor.matmul(v_ps, kvxT[:, dc, g * KT:(g + 1) * KT],
                                         wv[:, dc, :], start=dc == 0, stop=dc == 1)
                    nc.vector.tensor_copy(vones[:, g, 0:C], v_ps)
            # relocate gate rows 32:64 -> 0:32 (SBUF->SBUF DMA partition remap)
            nc.gpsimd.dma_start(gT[0:C, :], ghi[C:2 * C, :])

            # ---------------- attention ----------------
            with (
                tc.tile_pool(name="ps_s", bufs=3, space="PSUM") as ps_s,
                tc.tile_pool(name="ps_pv", bufs=2, space="PSUM") as ps_pv,
            ):
                for b in range(B):
                    pv = [ps_pv.tile([33, QT], f32, tag="pv", name=f"pv_{b}_{i}")
                          for i in range(2)]
                    for kt in range(NKT):
                        gk = b * NKT + kt
                        bp = biasp.tile([128, Q], bf16)
                        nc.sync.dma_start(bp, bpT_d[b, kt])
                        s = ps_s.tile([128, Q], f32, tag="s")
                        for jq in range(2):
                            nc.tensor.matmul(
                                s[:, jq * QT:(jq + 1) * QT],
                                kT[:, b * KL + kt * KT: b * KL + (kt + 1) * KT],
                                qT[:, b * Q + jq * QT: b * Q + (jq + 1) * QT],
                                start=True, stop=False)
                        for jq in range(2):
                            nc.tensor.matmul(
                                s[:, jq * QT:(jq + 1) * QT],
                                id128, bp[:, jq * QT:(jq + 1) * QT],
                                start=False, stop=True)
                        p = pp.tile([128, Q], bf16)
                        nc.scalar.activation(p, s, AF.Exp, bias=bm[:, gk:gk + 1])
                        for jq in range(2):
                            nc.tensor.matmul(
                                pv[jq][0:33, :], vones[:, gk, :],
                                p[:, jq * QT:(jq + 1) * QT],
                                start=kt == 0, stop=kt == NKT - 1)
                    for jq in range(2):
                        qsl = slice(b * Q + jq * QT, b * Q + (jq + 1) * QT)
                        # odn = (pv * 1.0) * [gate; 1]  (fused evict + gate
                        # mult; row 32 passes the denominator through)
                        nc.vector.scalar_tensor_tensor(
                            odn[:, qsl], pv[jq][0:33, :], 1.0,
                            gT[:, qsl], op0=ALU.mult, op1=ALU.mult)

            # ---------------- denominators -> per-partition columns ----------
            # SBUF->SBUF DMA walks dst (p, f) in order, consuming the source
            # row linearly: den_cols[p, f] = denom(q = p*32 + f).
            nc.gpsimd.dma_start(den_cols, odn[32:33, :])
            nc.vector.reciprocal(recip, den_cols)

            # ---------------- output projection ----------------
            # final tile f covers the strided q-set {p*32 + f}: lhsT columns
            # q = p*32+f, per-partition denominators recip[:, f], and a DRAM
            # AP with constant partition stride 32 rows.
            og_r = odn[0:C, :].rearrange("c (p f) -> c f p", f=NF)
            out_r = out_d[:, :].rearrange("(p f) d -> f p d", f=NF)
            with tc.tile_pool(name="ps_f", bufs=4, space="PSUM") as ps_f:
                for f in range(NF):
                    fo = ps_f.tile([128, D], f32, tag="f")
                    nc.tensor.matmul(fo, og_r[:, f, :], wo,
                                     start=True, stop=True)
                    ot = outp.tile([128, D], f32)
                    nc.vector.tensor_scalar_mul(ot, fo, recip[:, f:f + 1])
                    nc.sync.dma_start(out_r[f], ot)

    nc.finalize()
    return nc


def _get_nc():
    if "nc" not in _CACHE:
        _CACHE["nc"] = _build_nc()
    return _CACHE["nc"]


def _prep(inputs):
    q_x = np.asarray(inputs["q_x"], np.float32)
    kv_x = np.asarray(inputs["kv_x"], np.float32)
    bias_mask = np.asarray(inputs["bias_mask"], np.float32)
    bias_pair = np.asarray(inputs["bias_pair"], np.float32)
    wq = np.asarray(inputs["wq"], np.float32)
    wk = np.asarray(inputs["wk"], np.float32)
    wv = np.asarray(inputs["wv"], np.float32)
    wg = np.asarray(inputs["wg"], np.float32)
    bg = np.asarray(inputs["bg"], np.float32)
    wo = np.asarray(inputs["wo"], np.float32)

    qxT = np.ascontiguousarray(q_x.reshape(BQ, D).T).astype(_BF16).reshape(2, 128, BQ)
    kvxT = np.ascontiguousarray(kv_x.reshape(BK, D).T).astype(_BF16).reshape(2, 128, BK)
    bm = np.ascontiguousarray(
        bias_mask.reshape(B, NKT, KT).transpose(2, 0, 1)).astype(np.float32)
    bm = bm.reshape(128, B * NKT)
    id128 = np.eye(128, dtype=_BF16)
    sc = 1.0 / math.sqrt(C)

    in_maps = []
    for h in range(NCORES):
        csl = slice(h * C, (h + 1) * C)
        bpT = np.ascontiguousarray(
            bias_pair[:, h].transpose(0, 2, 1)).astype(_BF16)
        bpT = bpT.reshape(B, NKT, KT, Q)
        in_maps.append({
            "qxT": qxT,
            "kvxT": kvxT,
            "bpT": bpT,
            "bm": bm,
            "wqg": np.ascontiguousarray(
                np.concatenate([wq[:, csl] * sc, wg[:, csl]], axis=1)
            ).astype(_BF16).reshape(2, 128, 2 * C),
            "wk": np.ascontiguousarray(wk[:, csl]).astype(_BF16).reshape(2, 128, C),
            "wv": np.ascontiguousarray(wv[:, csl]).astype(_BF16).reshape(2, 128, C),
            "bg": np.concatenate(
                [np.zeros(C, np.float32), bg[csl].astype(np.float32)]
            ).reshape(2 * C, 1),
            "wo": np.ascontiguousarray(wo[csl, :]).astype(_BF16),
            "id128": id128,
        })
    return in_maps


def _run(inputs, trace=False, **kw):
    from concourse.bass_utils import run_bass_kernel_spmd

    in_maps = _prep(inputs)
    nc = _get_nc()
    r = run_bass_kernel_spmd(nc, in_maps, core_ids=list(range(NCORES)),
                             trace=trace, **kw)
    bo = np.asarray(inputs["bo"], np.float32)
    total = np.zeros((BQ, D), np.float32)
    for i in range(NCORES):
        total += r.results[i]["out"].reshape(BQ, D).astype(np.float32)
    total += bo
    return total.reshape(B, Q, D).astype(np.float32), r


def kernel(**inputs):
    out, _ = _run(inputs, trace=False)
    return out


# revision 8
# speedup vs baseline: 1.1190x; 1.0494x over previous
"""Fused biased-softmax attention (nn_Attention_55576876810478) on 8 TRN2 NeuronCores.

Tensor-parallel by head (H=8 -> 1 head/core): core h computes head h end to
end -- q/k/v/gate projections, scores with bias_mask+bias_pair, softmax,
P@V, sigmoid gate, and its 32 rows of the output projection -- producing a
partial [B*Q, D] output.  The host sums the 8 partials (the "all-reduce
after linear_o" of the sharding hint, done during unshard) and adds bo.

On-chip layout choices:
  * scores are computed TRANSPOSED, S^T[k, q], so the P@V contraction (over
    k) lands on the partition axis, and bias_mask (a function of k only)
    becomes a per-partition bias folded into the ACT exp instruction.
  * bias_pair arrives host-pre-transposed as bpT[b, kt, k%128, q] (bf16) and
    is accumulated into the scores PSUM with an identity matmul.
  * softmax denominators come for free from the P@V matmul: the stationary
    operand is [V | ones-columns], so row 32+t of the PV accumulator is
    sum_k P[k, q].  Denominators are transposed to [q-partition, 1] columns
    with a tiny K=8 identity matmul, and the divide is applied as a
    per-partition tensor_scalar multiply while evacuating the final matmul.
"""

import math

import ml_dtypes
import numpy as np

B, Q, KL, D, H, C = 4, 1024, 1024, 256, 8, 32
NCORES = 8
BQ = B * Q            # 4096 flattened q positions
BK = B * KL           # 4096 flattened k positions
QT = 512              # q-tile width (free dim of S^T)
KT = 128              # k-tile height (partition dim of S^T)
NQT = BQ // QT        # 8 (b, jq) q-tiles
NKT = KL // KT        # 8 k-tiles per batch
NVG = BK // KT        # 32 global k-tiles (v projection)
NF = BQ // 128        # 32 final output row-tiles

_BF16 = ml_dtypes.bfloat16
_CACHE = {}


def _build_nc():
    import concourse.bass as bass  # noqa: F401
    import concourse.mybir as mybir
    import concourse.tile as tile
    from concourse.bacc import Bacc

    bf16 = mybir.dt.bfloat16
    f32 = mybir.dt.float32
    AF = mybir.ActivationFunctionType
    ALU = mybir.AluOpType

    nc = Bacc(None, target_bir_lowering=False)

    qxT_d = nc.dram_tensor("qxT", [2, 128, BQ], bf16, kind="ExternalInput")
    kvxT_d = nc.dram_tensor("kvxT", [2, 128, BK], bf16, kind="ExternalInput")
    bpT_d = nc.dram_tensor("bpT", [B, NKT, KT, Q], bf16, kind="ExternalInput")
    wqg_d = nc.dram_tensor("wqg", [2, 128, 2 * C], bf16, kind="ExternalInput")
    wk_d = nc.dram_tensor("wk", [2, 128, C], bf16, kind="ExternalInput")
    wv_d = nc.dram_tensor("wv", [2, 128, C], bf16, kind="ExternalInput")
    bg_d = nc.dram_tensor("bg", [2 * C, 1], f32, kind="ExternalInput")
    wo_d = nc.dram_tensor("wo", [C, D], bf16, kind="ExternalInput")
    out_d = nc.dram_tensor("out", [BQ, D], f32, kind="ExternalOutput")

    with tile.TileContext(nc) as tc:
        with (
            tc.tile_pool(name="const", bufs=1) as const,
            tc.tile_pool(name="proj", bufs=1) as proj,
            tc.tile_pool(name="biasp", bufs=5) as biasp,
            tc.tile_pool(name="pp", bufs=4) as pp,
            tc.tile_pool(name="outp", bufs=3) as outp,
        ):
            # ---------------- constants / inputs ----------------
            qxT = const.tile([128, 2, BQ], bf16)
            kvxT = const.tile([128, 2, BK], bf16)
            for dc in range(2):
                nc.sync.dma_start(qxT[:, dc, :], qxT_d[dc])
                nc.sync.dma_start(kvxT[:, dc, :], kvxT_d[dc])
            wqg = const.tile([128, 2, 2 * C], bf16)
            wk = const.tile([128, 2, C], bf16)
            wv = const.tile([128, 2, C], bf16)
            for sb, dr in ((wqg, wqg_d), (wk, wk_d), (wv, wv_d)):
                for dc in range(2):
                    nc.sync.dma_start(sb[:, dc, :], dr[dc])
            bg = const.tile([2 * C, 1], f32)
            nc.sync.dma_start(bg, bg_d[:, :])
            wo = const.tile([C, D], bf16)
            nc.sync.dma_start(wo, wo_d[:, :])

            # persistent intermediates
            qT = proj.tile([C, BQ], bf16)        # [c, b*Q+q]
            ghi = proj.tile([2 * C, BQ], bf16)   # sigmoid out on partitions 32:64
            gT = proj.tile([33, BQ], bf16)       # sigmoid gate; row 32 = 1.0
            kT = proj.tile([C, BK], bf16)        # [c, b*K+k]
            vones = proj.tile([128, NVG, 33], bf16)  # [k%128, ktile, c|ones]
            odn = proj.tile([33, BQ], bf16)      # gated O^T (rows 0:32) + denom (row 32)
            den_cols = proj.tile([128, NF], f32)  # den_cols[p, f] = denom(q = p*32+f)
            recip = proj.tile([128, NF], f32)

            nc.vector.memset(vones, 1.0)
            nc.vector.memset(gT[32:33, :], 1.0)

            # ---------------- projections ----------------
            with tc.tile_pool(name="ps_proj", bufs=3, space="PSUM") as ps_pj:
                for j in range(NQT):
                    sl = slice(j * QT, (j + 1) * QT)
                    qg_ps = ps_pj.tile([2 * C, QT], f32, tag="m")
                    for dc in range(2):
                        nc.tensor.matmul(qg_ps, wqg[:, dc, :], qxT[:, dc, sl],
                                         start=dc == 0, stop=dc == 1)
                    nc.vector.tensor_copy(qT[:, sl], qg_ps[0:C, :])
                    nc.scalar.activation(ghi[C:2 * C, sl], qg_ps[C:2 * C, :],
                                         AF.Sigmoid, bias=bg[C:2 * C, :])
                    k_ps = ps_pj.tile([C, QT], f32, tag="m")
                    for dc in range(2):
                        nc.tensor.matmul(k_ps, wk[:, dc, :], kvxT[:, dc, sl],
                                         start=dc == 0, stop=dc == 1)
                    nc.vector.tensor_copy(kT[:, sl], k_ps)
                for g in range(NVG):
                    v_ps = ps_pj.tile([128, C], f32, tag="m")
                    for dc in range(2):
                        nc.tensor.matmul(v_ps, kvxT[:, dc, g * KT:(g + 1) * KT],
                                         wv[:, dc, :], start=dc == 0, stop=dc == 1)
                    nc.vector.tensor_copy(vones[:, g, 0:C], v_ps)
            # relocate gate rows 32:64 -> 0:32 (SBUF->SBUF DMA partition remap)
            nc.gpsimd.dma_start(gT[0:C, :], ghi[C:2 * C, :])

            # ---------------- attention ----------------
            with (
                tc.tile_pool(name="ps_s", bufs=3, space="PSUM") as ps_s,
                tc.tile_pool(name="ps_pv", bufs=2, space="PSUM") as ps_pv,
            ):
                for b in range(B):
                    pv = [ps_pv.tile([33, QT], f32, tag="pv", name=f"pv_{b}_{i}")
                          for i in range(2)]
                    for kt in range(NKT):
                        gk = b * NKT + kt
                        bp = biasp.tile([128, Q], bf16)
                        nc.sync.dma_start(bp, bpT_d[b, kt])
                        s = ps_s.tile([128, Q], f32, tag="s")
                        for jq in range(2):
                            nc.tensor.matmul(
                                s[:, jq * QT:(jq + 1) * QT],
                                kT[:, b * KL + kt * KT: b * KL + (kt + 1) * KT],
                                qT[:, b * Q + jq * QT: b * Q + (jq + 1) * QT],
                                start=True, stop=True)
                        praw = pp.tile([128, Q], bf16, tag="praw")
                        nc.scalar.activation(praw, s, AF.Exp)
                        p = pp.tile([128, Q], bf16, tag="p")
                        # P = exp(S) * exp(bias_pair + bias_mask); split the
                        # elementwise multiply across DVE and GpSimd
                        me = nc.vector if gk % 2 == 0 else nc.gpsimd
                        me.tensor_mul(p, praw, bp)
                        for jq in range(2):
                            nc.tensor.matmul(
                                pv[jq][0:33, :], vones[:, gk, :],
                                p[:, jq * QT:(jq + 1) * QT],
                                start=kt == 0, stop=kt == NKT - 1)
                    for jq in range(2):
                        qsl = slice(b * Q + jq * QT, b * Q + (jq + 1) * QT)
                        # odn = (pv * 1.0) * [gate; 1]  (fused evict + gate
                        # mult; row 32 passes the denominator through)
                        nc.vector.scalar_tensor_tensor(
                            odn[:, qsl], pv[jq][0:33, :], 1.0,
                            gT[:, qsl], op0=ALU.mult, op1=ALU.mult)

            # ---------------- denominators -> per-partition columns ----------
            # SBUF->SBUF DMA walks dst (p, f) in order, consuming the source
            # row linearly: den_cols[p, f] = denom(q = p*32 + f).
            nc.gpsimd.dma_start(den_cols, odn[32:33, :])
            nc.vector.reciprocal(recip, den_cols)

            # ---------------- output projection ----------------
            # final tile f covers the strided q-set {p*32 + f}: lhsT columns
            # q = p*32+f, per-partition denominators recip[:, f], and a DRAM
            # AP with constant partition stride 32 rows.
            og_r = odn[0:C, :].rearrange("c (p f) -> c f p", f=NF)
            out_r = out_d[:, :].rearrange("(p f) d -> f p d", f=NF)
            with tc.tile_pool(name="ps_f", bufs=4, space="PSUM") as ps_f:
                for f in range(NF):
                    fo = ps_f.tile([128, D], f32, tag="f")
                    nc.tensor.matmul(fo, og_r[:, f, :], wo,
                                     start=True, stop=True)
                    ot = outp.tile([128, D], f32)
                    nc.vector.tensor_scalar_mul(ot, fo, recip[:, f:f + 1])
                    nc.sync.dma_start(out_r[f], ot)

    nc.finalize()
    return nc


def _get_nc():
    if "nc" not in _CACHE:
        _CACHE["nc"] = _build_nc()
    return _CACHE["nc"]


def _prep(inputs):
    q_x = np.asarray(inputs["q_x"], np.float32)
    kv_x = np.asarray(inputs["kv_x"], np.float32)
    bias_mask = np.asarray(inputs["bias_mask"], np.float32)
    bias_pair = np.asarray(inputs["bias_pair"], np.float32)
    wq = np.asarray(inputs["wq"], np.float32)
    wk = np.asarray(inputs["wk"], np.float32)
    wv = np.asarray(inputs["wv"], np.float32)
    wg = np.asarray(inputs["wg"], np.float32)
    bg = np.asarray(inputs["bg"], np.float32)
    wo = np.asarray(inputs["wo"], np.float32)

    qxT = np.ascontiguousarray(q_x.reshape(BQ, D).T).astype(_BF16).reshape(2, 128, BQ)
    kvxT = np.ascontiguousarray(kv_x.reshape(BK, D).T).astype(_BF16).reshape(2, 128, BK)
    bmk = bias_mask.reshape(B, KL)  # varies along k only
    sc = 1.0 / math.sqrt(C)

    in_maps = []
    for h in range(NCORES):
        csl = slice(h * C, (h + 1) * C)
        bpT = np.exp(bias_pair[:, h].transpose(0, 2, 1)
                     + bmk[:, :, None]).astype(_BF16)
        bpT = bpT.reshape(B, NKT, KT, Q)
        in_maps.append({
            "qxT": qxT,
            "kvxT": kvxT,
            "bpT": bpT,
            "wqg": np.ascontiguousarray(
                np.concatenate([wq[:, csl] * sc, wg[:, csl]], axis=1)
            ).astype(_BF16).reshape(2, 128, 2 * C),
            "wk": np.ascontiguousarray(wk[:, csl]).astype(_BF16).reshape(2, 128, C),
            "wv": np.ascontiguousarray(wv[:, csl]).astype(_BF16).reshape(2, 128, C),
            "bg": np.concatenate(
                [np.zeros(C, np.float32), bg[csl].astype(np.float32)]
            ).reshape(2 * C, 1),
            "wo": np.ascontiguousarray(wo[csl, :]).astype(_BF16),
        })
    return in_maps


def _run(inputs, trace=False, **kw):
    from concourse.bass_utils import run_bass_kernel_spmd

    in_maps = _prep(inputs)
    nc = _get_nc()
    r = run_bass_kernel_spmd(nc, in_maps, core_ids=list(range(NCORES)),
                             trace=trace, **kw)
    bo = np.asarray(inputs["bo"], np.float32)
    total = np.zeros((BQ, D), np.float32)
    for i in range(NCORES):
        total += r.results[i]["out"].reshape(BQ, D).astype(np.float32)
    total += bo
    return total.reshape(B, Q, D).astype(np.float32), r


def kernel(**inputs):
    out, _ = _run(inputs, trace=False)
    return out


# revision 11
# speedup vs baseline: 1.2298x; 1.0990x over previous
"""Fused biased-softmax attention (nn_Attention_55576876810478) on 8 TRN2 NeuronCores.

Tensor-parallel by head (H=8 -> 1 head/core): core h computes head h end to
end -- q/k/v/gate projections, scores with bias_mask+bias_pair, softmax,
P@V, sigmoid gate, and its 32 rows of the output projection -- producing a
partial [B*Q, D] output.  The host sums the 8 partials (the "all-reduce
after linear_o" of the sharding hint, done during unshard) and adds bo.

On-chip layout choices:
  * scores are computed TRANSPOSED, S^T[k, q], so the P@V contraction (over
    k) lands on the partition axis, and bias_mask (a function of k only)
    becomes a per-partition bias folded into the ACT exp instruction.
  * bias_pair arrives host-pre-transposed as bpT[b, kt, k%128, q] (bf16) and
    is accumulated into the scores PSUM with an identity matmul.
  * softmax denominators come for free from the P@V matmul: the stationary
    operand is [V | ones-columns], so row 32+t of the PV accumulator is
    sum_k P[k, q].  Denominators are transposed to [q-partition, 1] columns
    with a tiny K=8 identity matmul, and the divide is applied as a
    per-partition tensor_scalar multiply while evacuating the final matmul.
"""

import math

import ml_dtypes
import numpy as np

B, Q, KL, D, H, C = 4, 1024, 1024, 256, 8, 32
NCORES = 8
BQ = B * Q            # 4096 flattened q positions
BK = B * KL           # 4096 flattened k positions
QT = 512              # q-tile width (free dim of S^T)
KT = 128              # k-tile height (partition dim of S^T)
NQT = BQ // QT        # 8 (b, jq) q-tiles
NKT = KL // KT        # 8 k-tiles per batch
NVG = BK // KT        # 32 global k-tiles (v projection)
NF = BQ // 128        # 32 final output row-tiles

_BF16 = ml_dtypes.bfloat16
_CACHE = {}


def _build_nc():
    import concourse.bass as bass  # noqa: F401
    import concourse.mybir as mybir
    import concourse.tile as tile
    from concourse.bacc import Bacc

    bf16 = mybir.dt.bfloat16
    f32 = mybir.dt.float32
    AF = mybir.ActivationFunctionType
    ALU = mybir.AluOpType

    nc = Bacc(None, target_bir_lowering=False)

    qxT_d = nc.dram_tensor("qxT", [2, 128, BQ], bf16, kind="ExternalInput")
    kvxT_d = nc.dram_tensor("kvxT", [2, 128, BK], bf16, kind="ExternalInput")
    bpT_d = nc.dram_tensor("bpT", [B, NKT, KT, Q], bf16, kind="ExternalInput")
    bm_d = nc.dram_tensor("bm", [128, B * NKT], f32, kind="ExternalInput")
    wqg_d = nc.dram_tensor("wqg", [2, 128, 2 * C], bf16, kind="ExternalInput")
    wk_d = nc.dram_tensor("wk", [2, 128, C], bf16, kind="ExternalInput")
    wv_d = nc.dram_tensor("wv", [2, 128, C], bf16, kind="ExternalInput")
    bg_d = nc.dram_tensor("bg", [2 * C, 1], f32, kind="ExternalInput")
    wo_d = nc.dram_tensor("wo", [C, D], bf16, kind="ExternalInput")
    id128_d = nc.dram_tensor("id128", [128, 128], bf16, kind="ExternalInput")
    out_d = nc.dram_tensor("out", [BQ, D], f32, kind="ExternalOutput")

    with tile.TileContext(nc) as tc:
        with (
            tc.tile_pool(name="const", bufs=1) as const,
            tc.tile_pool(name="proj", bufs=1) as proj,
            tc.tile_pool(name="biasp", bufs=5) as biasp,
            tc.tile_pool(name="pp", bufs=4) as pp,
            tc.tile_pool(name="outp", bufs=3) as outp,
        ):
            # ---------------- constants / inputs ----------------
            qxT = const.tile([128, 2, BQ], bf16)
            kvxT = const.tile([128, 2, BK], bf16)
            for dc in range(2):
                nc.sync.dma_start(qxT[:, dc, :], qxT_d[dc])
                nc.sync.dma_start(kvxT[:, dc, :], kvxT_d[dc])
            wqg = const.tile([128, 2, 2 * C], bf16)
            wk = const.tile([128, 2, C], bf16)
            wv = const.tile([128, 2, C], bf16)
            for sb, dr in ((wqg, wqg_d), (wk, wk_d), (wv, wv_d)):
                for dc in range(2):
                    nc.sync.dma_start(sb[:, dc, :], dr[dc])
            bm = const.tile([128, B * NKT], f32)
            nc.sync.dma_start(bm, bm_d[:, :])
            bg = const.tile([2 * C, 1], f32)
            nc.sync.dma_start(bg, bg_d[:, :])
            wo = const.tile([C, D], bf16)
            nc.sync.dma_start(wo, wo_d[:, :])
            id128 = const.tile([128, 128], bf16)
            nc.sync.dma_start(id128, id128_d[:, :])

            # persistent intermediates
            qT = proj.tile([C, BQ], bf16)        # [c, b*Q+q]
            qT_r = proj.tile([128, BQ], bf16)    # qT replicated at 4 row groups
            kT_g = proj.tile([128, NVG // 4, KT], bf16)  # group kt%4, block kt//4
            og4 = proj.tile([128, BQ], bf16)     # gated O^T replicated 4 groups
            wo_r = proj.tile([128, D], bf16)     # wo replicated at 4 groups
            ghi = proj.tile([2 * C, BQ], bf16)   # sigmoid out on partitions 32:64
            gT = proj.tile([33, BQ], bf16)       # sigmoid gate; row 32 = 1.0
            kT = proj.tile([C, BK], bf16)        # [c, b*K+k]
            vones = proj.tile([128, NVG, 33], bf16)  # [k%128, ktile, c|ones]
            odn = proj.tile([33, BQ], bf16)      # gated O^T (rows 0:32) + denom (row 32)
            den_cols = proj.tile([128, NF], f32)  # den_cols[p, f] = denom(q = p*32+f)
            recip = proj.tile([128, NF], f32)

            nc.vector.memset(vones, 1.0)
            nc.vector.memset(gT[32:33, :], 1.0)

            # ---------------- projections ----------------
            with tc.tile_pool(name="ps_proj", bufs=3, space="PSUM") as ps_pj:
                for j in range(NQT):
                    sl = slice(j * QT, (j + 1) * QT)
                    qg_ps = ps_pj.tile([2 * C, QT], f32, tag="m")
                    for dc in range(2):
                        nc.tensor.matmul(qg_ps, wqg[:, dc, :], qxT[:, dc, sl],
                                         start=dc == 0, stop=dc == 1)
                    nc.vector.tensor_copy(qT[:, sl], qg_ps[0:C, :])
                    nc.scalar.activation(ghi[C:2 * C, sl], qg_ps[C:2 * C, :],
                                         AF.Sigmoid, bias=bg[C:2 * C, :])
                    k_ps = ps_pj.tile([C, QT], f32, tag="m")
                    for dc in range(2):
                        nc.tensor.matmul(k_ps, wk[:, dc, :], kvxT[:, dc, sl],
                                         start=dc == 0, stop=dc == 1)
                    nc.vector.tensor_copy(kT[:, sl], k_ps)
                for g in range(NVG):
                    v_ps = ps_pj.tile([128, C], f32, tag="m")
                    for dc in range(2):
                        nc.tensor.matmul(v_ps, kvxT[:, dc, g * KT:(g + 1) * KT],
                                         wv[:, dc, :], start=dc == 0, stop=dc == 1)
                    nc.vector.tensor_copy(vones[:, g, 0:C], v_ps)
            # relocate gate rows 32:64 -> 0:32 (SBUF->SBUF DMA partition remap)
            nc.gpsimd.dma_start(gT[0:C, :], ghi[C:2 * C, :])
            # scatter kT so k-tile gt lives at partition group gt%4, col block
            # gt//4  (enables 4x row-packed score matmuls)
            kT_v = kT.rearrange("c (cb g k) -> c cb g k", g=4, k=KT)
            for g4 in range(4):
                nc.gpsimd.dma_start(kT_g[32 * g4:32 * (g4 + 1), :, :],
                                    kT_v[:, :, g4, :])
            # replicate qT at all 4 row groups
            for g4 in range(4):
                nc.gpsimd.dma_start(qT_r[32 * g4:32 * (g4 + 1), :], qT)

            # ---------------- attention ----------------
            with (
                tc.tile_pool(name="ps_s", bufs=5, space="PSUM") as ps_s,
                tc.tile_pool(name="ps_pv", bufs=2, space="PSUM") as ps_pv,
            ):
                for b in range(B):
                    pv = [ps_pv.tile([33, QT], f32, tag="pv", name=f"pv_{b}_{i}")
                          for i in range(2)]
                    for pk in range(2):
                        bps = []
                        for g4 in range(4):
                            kt = 4 * pk + g4
                            bp = biasp.tile([128, Q], bf16, tag="bias",
                                            name=f"bp_{b}_{pk}_{g4}")
                            nc.sync.dma_start(bp, bpT_d[b, kt])
                            bps.append(bp)
                        for jq in range(2):
                            qsl = slice(b * Q + jq * QT, b * Q + (jq + 1) * QT)
                            sb = []
                            # 4x row-packed score matmuls (K=32 each)
                            for g4 in range(4):
                                s = ps_s.tile([128, QT], f32, tag="s",
                                              name=f"s_{b}_{pk}_{jq}_{g4}")
                                nc.tensor.matmul(
                                    s, kT_g[32 * g4:32 * (g4 + 1), 2 * b + pk, :],
                                    qT_r[32 * g4:32 * (g4 + 1), qsl],
                                    start=True, stop=False,
                                    tile_position=(32 * g4, 0))
                                sb.append(s)
                            for g4 in range(4):
                                kt = 4 * pk + g4
                                gk = b * NKT + kt
                                nc.tensor.matmul(
                                    sb[g4], id128,
                                    bps[g4][:, jq * QT:(jq + 1) * QT],
                                    start=False, stop=True)
                                p = pp.tile([128, QT], bf16, tag="p")
                                nc.scalar.activation(p, sb[g4], AF.Exp,
                                                     bias=bm[:, gk:gk + 1])
                                nc.tensor.matmul(
                                    pv[jq][0:33, :], vones[:, gk, :], p,
                                    start=kt == 0, stop=kt == NKT - 1)
                    for jq in range(2):
                        qsl = slice(b * Q + jq * QT, b * Q + (jq + 1) * QT)
                        # odn = (pv * 1.0) * [gate; 1]  (fused evict + gate
                        # mult; row 32 passes the denominator through)
                        nc.vector.scalar_tensor_tensor(
                            odn[:, qsl], pv[jq][0:33, :], 1.0,
                            gT[:, qsl], op0=ALU.mult, op1=ALU.mult)

            # ---------------- denominators -> per-partition columns ----------
            # SBUF->SBUF DMA walks dst (p, f) in order, consuming the source
            # row linearly: den_cols[p, f] = denom(q = p*32 + f).
            nc.gpsimd.dma_start(den_cols, odn[32:33, :])
            nc.vector.reciprocal(recip, den_cols)

            # ---------------- output projection ----------------
            # final tile f covers the strided q-set {p*32 + f}: lhsT columns
            # q = p*32+f, per-partition denominators recip[:, f], and a DRAM
            # AP with constant partition stride 32 rows.
            for g4 in range(4):
                nc.gpsimd.dma_start(og4[32 * g4:32 * (g4 + 1), :], odn[0:C, :])
                nc.gpsimd.dma_start(wo_r[32 * g4:32 * (g4 + 1), :], wo)
            og4_r = og4.rearrange("c (p f) -> c f p", f=NF)
            out_r = out_d[:, :].rearrange("(p f) d -> f p d", f=NF)
            with tc.tile_pool(name="ps_f", bufs=8, space="PSUM") as ps_f:
                for fp in range(NF // 4):
                    for g4 in range(4):
                        f = fp * 4 + g4
                        fo = ps_f.tile([128, D], f32, tag="f",
                                       name=f"fo_{f}")
                        nc.tensor.matmul(
                            fo, og4_r[32 * g4:32 * (g4 + 1), f, :],
                            wo_r[32 * g4:32 * (g4 + 1), :],
                            start=True, stop=True,
                            tile_position=(32 * g4, 0))
                        ot = outp.tile([128, D], f32, tag="ot", name=f"ot_{f}")
                        nc.scalar.activation(ot, fo, AF.Copy,
                                             scale=recip[:, f:f + 1])
                        nc.sync.dma_start(out_r[f], ot)

    nc.finalize()
    return nc


def _get_nc():
    if "nc" not in _CACHE:
        _CACHE["nc"] = _build_nc()
    return _CACHE["nc"]


def _prep(inputs):
    q_x = np.asarray(inputs["q_x"], np.float32)
    kv_x = np.asarray(inputs["kv_x"], np.float32)
    bias_mask = np.asarray(inputs["bias_mask"], np.float32)
    bias_pair = np.asarray(inputs["bias_pair"], np.float32)
    wq = np.asarray(inputs["wq"], np.float32)
    wk = np.asarray(inputs["wk"], np.float32)
    wv = np.asarray(inputs["wv"], np.float32)
    wg = np.asarray(inputs["wg"], np.float32)
    bg = np.asarray(inputs["bg"], np.float32)
    wo = np.asarray(inputs["wo"], np.float32)

    qxT = np.ascontiguousarray(q_x.reshape(BQ, D).T).astype(_BF16).reshape(2, 128, BQ)
    kvxT = np.ascontiguousarray(kv_x.reshape(BK, D).T).astype(_BF16).reshape(2, 128, BK)
    bm = np.ascontiguousarray(
        bias_mask.reshape(B, NKT, KT).transpose(2, 0, 1)).astype(np.float32)
    bm = bm.reshape(128, B * NKT)
    id128 = np.eye(128, dtype=_BF16)
    sc = 1.0 / math.sqrt(C)

    in_maps = []
    for h in range(NCORES):
        csl = slice(h * C, (h + 1) * C)
        bpT = np.ascontiguousarray(
            bias_pair[:, h].transpose(0, 2, 1)).astype(_BF16)
        bpT = bpT.reshape(B, NKT, KT, Q)
        in_maps.append({
            "qxT": qxT,
            "kvxT": kvxT,
            "bpT": bpT,
            "bm": bm,
            "wqg": np.ascontiguousarray(
                np.concatenate([wq[:, csl] * sc, wg[:, csl]], axis=1)
            ).astype(_BF16).reshape(2, 128, 2 * C),
            "wk": np.ascontiguousarray(wk[:, csl]).astype(_BF16).reshape(2, 128, C),
            "wv": np.ascontiguousarray(wv[:, csl]).astype(_BF16).reshape(2, 128, C),
            "bg": np.concatenate(
                [np.zeros(C, np.float32), bg[csl].astype(np.float32)]
            ).reshape(2 * C, 1),
            "wo": np.ascontiguousarray(wo[csl, :]).astype(_BF16),
            "id128": id128,
        })
    return in_maps


def _run(inputs, trace=False, **kw):
    from concourse.bass_utils import run_bass_kernel_spmd

    in_maps = _prep(inputs)
    nc = _get_nc()
    r = run_bass_kernel_spmd(nc, in_maps, core_ids=list(range(NCORES)),
                             trace=trace, **kw)
    bo = np.asarray(inputs["bo"], np.float32)
    total = np.zeros((BQ, D), np.float32)
    for i in range(NCORES):
        total += r.results[i]["out"].reshape(BQ, D).astype(np.float32)
    total += bo
    return total.reshape(B, Q, D).astype(np.float32), r


def kernel(**inputs):
    out, _ = _run(inputs, trace=False)
    return out


# revision 12
# speedup vs baseline: 1.3082x; 1.0638x over previous
"""Fused biased-softmax attention (nn_Attention_55576876810478) on 8 TRN2 NeuronCores.

Tensor-parallel by head (H=8 -> 1 head/core): core h computes head h end to
end -- q/k/v/gate projections, scores with bias_mask+bias_pair, softmax,
P@V, sigmoid gate, and its 32 rows of the output projection -- producing a
partial [B*Q, D] output.  The host sums the 8 partials (the "all-reduce
after linear_o" of the sharding hint, done during unshard) and adds bo.

On-chip layout choices:
  * scores are computed TRANSPOSED, S^T[k, q], so the P@V contraction (over
    k) lands on the partition axis, and bias_mask (a function of k only)
    becomes a per-partition bias folded into the ACT exp instruction.
  * bias_pair arrives host-pre-transposed as bpT[b, kt, k%128, q] (bf16) and
    is accumulated into the scores PSUM with an identity matmul.
  * softmax denominators come for free from the P@V matmul: the stationary
    operand is [V | ones-columns], so row 32+t of the PV accumulator is
    sum_k P[k, q].  Denominators are transposed to [q-partition, 1] columns
    with a tiny K=8 identity matmul, and the divide is applied as a
    per-partition tensor_scalar multiply while evacuating the final matmul.
"""

import math

import ml_dtypes
import numpy as np

B, Q, KL, D, H, C = 4, 1024, 1024, 256, 8, 32
NCORES = 8
BQ = B * Q            # 4096 flattened q positions
BK = B * KL           # 4096 flattened k positions
QT = 512              # q-tile width (free dim of S^T)
KT = 128              # k-tile height (partition dim of S^T)
NQT = BQ // QT        # 8 (b, jq) q-tiles
NKT = KL // KT        # 8 k-tiles per batch
NVG = BK // KT        # 32 global k-tiles (v projection)
NF = BQ // 128        # 32 final output row-tiles

_BF16 = ml_dtypes.bfloat16
_CACHE = {}


def _build_nc():
    import concourse.bass as bass  # noqa: F401
    import concourse.mybir as mybir
    import concourse.tile as tile
    from concourse.bacc import Bacc

    bf16 = mybir.dt.bfloat16
    f32 = mybir.dt.float32
    AF = mybir.ActivationFunctionType
    ALU = mybir.AluOpType

    nc = Bacc(None, target_bir_lowering=False)

    qxT_d = nc.dram_tensor("qxT", [2, 128, BQ], bf16, kind="ExternalInput")
    kvxT_d = nc.dram_tensor("kvxT", [2, 128, BK], bf16, kind="ExternalInput")
    bpT_d = nc.dram_tensor("bpT", [B, NKT, KT, Q], bf16, kind="ExternalInput")
    bm_d = nc.dram_tensor("bm", [128, B * NKT], f32, kind="ExternalInput")
    wqg_d = nc.dram_tensor("wqg", [2, 128, 2 * C], bf16, kind="ExternalInput")
    wk_d = nc.dram_tensor("wk", [2, 128, C], bf16, kind="ExternalInput")
    wv_d = nc.dram_tensor("wv", [2, 128, C], bf16, kind="ExternalInput")
    bg_d = nc.dram_tensor("bg", [2 * C, 1], f32, kind="ExternalInput")
    wo_d = nc.dram_tensor("wo", [C, D], bf16, kind="ExternalInput")
    id128_d = nc.dram_tensor("id128", [128, 128], bf16, kind="ExternalInput")
    out_d = nc.dram_tensor("out", [BQ, D], f32, kind="ExternalOutput")

    with tile.TileContext(nc) as tc:
        with (
            tc.tile_pool(name="const", bufs=1) as const,
            tc.tile_pool(name="proj", bufs=1) as proj,
            tc.tile_pool(name="biasp", bufs=9) as biasp,
            tc.tile_pool(name="pp", bufs=8) as pp,
            tc.tile_pool(name="outp", bufs=3) as outp,
        ):
            # ---------------- constants / inputs ----------------
            qxT = const.tile([128, 2, BQ], bf16)
            kvxT = const.tile([128, 2, BK], bf16)
            for dc in range(2):
                nc.sync.dma_start(qxT[:, dc, :], qxT_d[dc])
                nc.sync.dma_start(kvxT[:, dc, :], kvxT_d[dc])
            wqg = const.tile([128, 2, 2 * C], bf16)
            wk = const.tile([128, 2, C], bf16)
            wv = const.tile([128, 2, C], bf16)
            for sb, dr in ((wqg, wqg_d), (wk, wk_d), (wv, wv_d)):
                for dc in range(2):
                    nc.sync.dma_start(sb[:, dc, :], dr[dc])
            bm = const.tile([128, B * NKT], f32)
            nc.sync.dma_start(bm, bm_d[:, :])
            bg = const.tile([2 * C, 1], f32)
            nc.sync.dma_start(bg, bg_d[:, :])
            wo = const.tile([C, D], bf16)
            nc.sync.dma_start(wo, wo_d[:, :])
            id128 = const.tile([128, 128], bf16)
            nc.sync.dma_start(id128, id128_d[:, :])

            # persistent intermediates
            qT = proj.tile([C, BQ], bf16)        # [c, b*Q+q]
            qT_r = proj.tile([128, BQ], bf16)    # qT replicated at 4 row groups
            kT_g = proj.tile([128, NVG // 4, KT], bf16)  # group kt%4, block kt//4
            og4 = proj.tile([128, BQ], bf16)     # gated O^T replicated 4 groups
            wo_r = proj.tile([128, D], bf16)     # wo replicated at 4 groups
            ghi = proj.tile([2 * C, BQ], bf16)   # sigmoid out on partitions 32:64
            gT = proj.tile([33, BQ], bf16)       # sigmoid gate; row 32 = 1.0
            kT = proj.tile([C, BK], bf16)        # [c, b*K+k]
            vones = proj.tile([128, NVG, 33], bf16)  # [k%128, ktile, c|ones]
            odn = proj.tile([33, BQ], bf16)      # gated O^T (rows 0:32) + denom (row 32)
            den_cols = proj.tile([128, NF], f32)  # den_cols[p, f] = denom(q = p*32+f)
            recip = proj.tile([128, NF], f32)

            nc.vector.memset(vones, 1.0)
            nc.vector.memset(gT[32:33, :], 1.0)

            # ---------------- projections ----------------
            with tc.tile_pool(name="ps_proj", bufs=3, space="PSUM") as ps_pj:
                for j in range(NQT):
                    sl = slice(j * QT, (j + 1) * QT)
                    qg_ps = ps_pj.tile([2 * C, QT], f32, tag="m")
                    for dc in range(2):
                        nc.tensor.matmul(qg_ps, wqg[:, dc, :], qxT[:, dc, sl],
                                         start=dc == 0, stop=dc == 1)
                    nc.vector.tensor_copy(qT[:, sl], qg_ps[0:C, :])
                    # sigmoid(x) = 0.5*tanh(0.5x) + 0.5 -- tanh lives in the
                    # same ACT table set as exp (one table load total)
                    nc.scalar.activation(ghi[C:2 * C, sl], qg_ps[C:2 * C, :],
                                         AF.Tanh, bias=bg[C:2 * C, :],
                                         scale=0.5)
                    nc.vector.tensor_scalar(ghi[C:2 * C, sl],
                                            ghi[C:2 * C, sl], 0.5, 0.5,
                                            op0=ALU.mult, op1=ALU.add)
                    k_ps = ps_pj.tile([C, QT], f32, tag="m")
                    for dc in range(2):
                        nc.tensor.matmul(k_ps, wk[:, dc, :], kvxT[:, dc, sl],
                                         start=dc == 0, stop=dc == 1)
                    nc.vector.tensor_copy(kT[:, sl], k_ps)
                    # prefetch this chunk's share of the kT / qT group layouts
                    for g4 in range(4):
                        nc.gpsimd.dma_start(
                            kT_g[32 * g4:32 * (g4 + 1), j, :],
                            kT[:, (4 * j + g4) * KT:(4 * j + g4 + 1) * KT])
                        nc.gpsimd.dma_start(
                            qT_r[32 * g4:32 * (g4 + 1), sl], qT[:, sl])
                for g in range(NVG):
                    v_ps = ps_pj.tile([128, C], f32, tag="m")
                    for dc in range(2):
                        nc.tensor.matmul(v_ps, kvxT[:, dc, g * KT:(g + 1) * KT],
                                         wv[:, dc, :], start=dc == 0, stop=dc == 1)
                    nc.vector.tensor_copy(vones[:, g, 0:C], v_ps)
            # relocate gate rows 32:64 -> 0:32 (SBUF->SBUF DMA partition remap)
            nc.gpsimd.dma_start(gT[0:C, :], ghi[C:2 * C, :])

            # ---------------- attention ----------------
            with (
                tc.tile_pool(name="ps_s", bufs=6, space="PSUM") as ps_s,
                tc.tile_pool(name="ps_pv", bufs=2, space="PSUM") as ps_pv,
            ):
                for b in range(B):
                    pv = [ps_pv.tile([33, QT], f32, tag="pv", name=f"pv_{b}_{i}")
                          for i in range(2)]
                    for pk in range(2):
                        bps = []
                        for g4 in range(4):
                            kt = 4 * pk + g4
                            bp = biasp.tile([128, Q], bf16, tag="bias",
                                            name=f"bp_{b}_{pk}_{g4}")
                            nc.sync.dma_start(bp, bpT_d[b, kt])
                            bps.append(bp)
                        for jq in range(2):
                            qsl = slice(b * Q + jq * QT, b * Q + (jq + 1) * QT)
                            sb = []
                            # 4x row-packed score matmuls (K=32 each)
                            for g4 in range(4):
                                s = ps_s.tile([128, QT], f32, tag="s",
                                              name=f"s_{b}_{pk}_{jq}_{g4}")
                                nc.tensor.matmul(
                                    s, kT_g[32 * g4:32 * (g4 + 1), 2 * b + pk, :],
                                    qT_r[32 * g4:32 * (g4 + 1), qsl],
                                    start=True, stop=False,
                                    tile_position=(32 * g4, 0))
                                sb.append(s)
                            for g4 in range(4):
                                kt = 4 * pk + g4
                                gk = b * NKT + kt
                                nc.tensor.matmul(
                                    sb[g4], id128,
                                    bps[g4][:, jq * QT:(jq + 1) * QT],
                                    start=False, stop=True)
                                p = pp.tile([128, QT], bf16, tag="p")
                                nc.scalar.activation(p, sb[g4], AF.Exp,
                                                     bias=bm[:, gk:gk + 1])
                                nc.tensor.matmul(
                                    pv[jq][0:33, :], vones[:, gk, :], p,
                                    start=kt == 0, stop=kt == NKT - 1)
                    for jq in range(2):
                        qsl = slice(b * Q + jq * QT, b * Q + (jq + 1) * QT)
                        # odn = (pv * 1.0) * [gate; 1]  (fused evict + gate
                        # mult; row 32 passes the denominator through)
                        nc.vector.scalar_tensor_tensor(
                            odn[:, qsl], pv[jq][0:33, :], 1.0,
                            gT[:, qsl], op0=ALU.mult, op1=ALU.mult)
                        for g4 in range(4):
                            nc.gpsimd.dma_start(
                                og4[32 * g4:32 * (g4 + 1), qsl],
                                odn[0:C, qsl])

            # ---------------- denominators -> per-partition columns ----------
            # SBUF->SBUF DMA walks dst (p, f) in order, consuming the source
            # row linearly: den_cols[p, f] = denom(q = p*32 + f).
            nc.gpsimd.dma_start(den_cols, odn[32:33, :])
            nc.vector.reciprocal(recip, den_cols)

            # ---------------- output projection ----------------
            # final tile f covers the strided q-set {p*32 + f}: lhsT columns
            # q = p*32+f, per-partition denominators recip[:, f], and a DRAM
            # AP with constant partition stride 32 rows.
            for g4 in range(4):
                nc.gpsimd.dma_start(wo_r[32 * g4:32 * (g4 + 1), :], wo)
            og4_r = og4.rearrange("c (p f) -> c f p", f=NF)
            out_r = out_d[:, :].rearrange("(p f) d -> f p d", f=NF)
            with tc.tile_pool(name="ps_f", bufs=8, space="PSUM") as ps_f:
                for fp in range(NF // 4):
                    for g4 in range(4):
                        f = fp * 4 + g4
                        fo = ps_f.tile([128, D], f32, tag="f",
                                       name=f"fo_{f}")
                        nc.tensor.matmul(
                            fo, og4_r[32 * g4:32 * (g4 + 1), f, :],
                            wo_r[32 * g4:32 * (g4 + 1), :],
                            start=True, stop=True,
                            tile_position=(32 * g4, 0))
                        ot = outp.tile([128, D], f32, tag="ot", name=f"ot_{f}")
                        nc.vector.tensor_scalar_mul(ot, fo, recip[:, f:f + 1])
                        nc.sync.dma_start(out_r[f], ot)

    nc.finalize()
    return nc


def _get_nc():
    if "nc" not in _CACHE:
        _CACHE["nc"] = _build_nc()
    return _CACHE["nc"]


def _prep(inputs):
    q_x = np.asarray(inputs["q_x"], np.float32)
    kv_x = np.asarray(inputs["kv_x"], np.float32)
    bias_mask = np.asarray(inputs["bias_mask"], np.float32)
    bias_pair = np.asarray(inputs["bias_pair"], np.float32)
    wq = np.asarray(inputs["wq"], np.float32)
    wk = np.asarray(inputs["wk"], np.float32)
    wv = np.asarray(inputs["wv"], np.float32)
    wg = np.asarray(inputs["wg"], np.float32)
    bg = np.asarray(inputs["bg"], np.float32)
    wo = np.asarray(inputs["wo"], np.float32)

    qxT = np.ascontiguousarray(q_x.reshape(BQ, D).T).astype(_BF16).reshape(2, 128, BQ)
    kvxT = np.ascontiguousarray(kv_x.reshape(BK, D).T).astype(_BF16).reshape(2, 128, BK)
    bm = np.ascontiguousarray(
        bias_mask.reshape(B, NKT, KT).transpose(2, 0, 1)).astype(np.float32)
    bm = bm.reshape(128, B * NKT)
    id128 = np.eye(128, dtype=_BF16)
    sc = 1.0 / math.sqrt(C)

    in_maps = []
    for h in range(NCORES):
        csl = slice(h * C, (h + 1) * C)
        bpT = np.ascontiguousarray(
            bias_pair[:, h].transpose(0, 2, 1)).astype(_BF16)
        bpT = bpT.reshape(B, NKT, KT, Q)
        in_maps.append({
            "qxT": qxT,
            "kvxT": kvxT,
            "bpT": bpT,
            "bm": bm,
            "wqg": np.ascontiguousarray(
                np.concatenate([wq[:, csl] * sc, wg[:, csl]], axis=1)
            ).astype(_BF16).reshape(2, 128, 2 * C),
            "wk": np.ascontiguousarray(wk[:, csl]).astype(_BF16).reshape(2, 128, C),
            "wv": np.ascontiguousarray(wv[:, csl]).astype(_BF16).reshape(2, 128, C),
            "bg": np.concatenate(
                [np.zeros(C, np.float32), 0.5 * bg[csl].astype(np.float32)]
            ).reshape(2 * C, 1),
            "wo": np.ascontiguousarray(wo[csl, :]).astype(_BF16),
            "id128": id128,
        })
    return in_maps


def _run(inputs, trace=False, **kw):
    from concourse.bass_utils import run_bass_kernel_spmd

    in_maps = _prep(inputs)
    nc = _get_nc()
    r = run_bass_kernel_spmd(nc, in_maps, core_ids=list(range(NCORES)),
                             trace=trace, **kw)
    bo = np.asarray(inputs["bo"], np.float32)
    total = np.zeros((BQ, D), np.float32)
    for i in range(NCORES):
        total += r.results[i]["out"].reshape(BQ, D).astype(np.float32)
    total += bo
    return total.reshape(B, Q, D).astype(np.float32), r


def kernel(**inputs):
    out, _ = _run(inputs, trace=False)
    return out
